# revision 1
# baseline (speedup 1.0000x reference)
"""Trainium2 Bass kernel for nn_MoEConnectionProcessor.

Data-parallel over cells: 8 cores x 2560 padded cells (19683 real).
Per core the cell range is processed in 40 "superblocks" of 64 cells
(= 13 subtiles of 128 edges, since 64*26 = 13*128 = 1664).

Layout strategy:
  - neighbor data loaded twice in bf16: natural [edge, d] tiles (for
    masked-aggregation matmuls, contract over edges) and DMA-transposed
    [d, edge] tiles (stationary operand for the per-edge message
    projection, giving natural-layout messages in PSUM).
  - all "second stage" activations live transposed [d, cell]; biases
    become per-partition ACT bias vectors there.
  - masked sums (mask = conn_type compare) are PE matmuls whose moving
    operand is a per-edge-scaled staircase matrix built in bulk on DVE.
  - 1/count normalization is applied at aggregate-evacuation time via a
    PE one-hot broadcast (bf16 hi+lo for fp32-grade accuracy).
"""

import numpy as np
import ml_dtypes
from contextlib import ExitStack

N_CELLS, K, D, HG = 19683, 26, 128, 64
NCORES = 8
NS = 2560                 # padded cells per core
SBC = 64                  # cells per superblock
NSB = NS // SBC           # 40 superblocks
NSUB = 13                 # subtiles (128 edges) per superblock
EPB = NSUB * 128          # 1664 edges per superblock
E = NS * K                # 66560 edges per core
NSUBT = NS * K // 128     # 520 subtiles per core
CHUNK = 512
NCHUNK = NS // CHUNK      # 5
SB_PER_CHUNK = CHUNK // SBC  # 8
CNF_STEPS, DTC = 3, 0.1

bf16 = ml_dtypes.bfloat16


def _staircase_consts():
    """Per-class (subtile position within superblock) staircase matrices."""
    # cb[chi]: first local cell of subtile chi; j = cell_local - cb in [0, 6)
    S6 = np.zeros((13, 128, 6), np.float32)
    S64T = np.zeros((13, 64, 128), np.float32)
    cbs = []
    for chi in range(13):
        cb = (chi * 128) // K
        cbs.append(cb)
        for p in range(128):
            cl = (chi * 128 + p) // K     # local cell 0..63
            S6[chi, p, cl - cb] = 1.0
            S64T[chi, cl, p] = 1.0
    return S6, S64T, cbs


S6_CLS, S64T_CLS, CB_LOC = _staircase_consts()


def _consts():
    c = {}
    # S6_big [128, 520*6], S12_big [128, 520*12] tiled over all subtiles
    s6 = np.tile(S6_CLS.transpose(1, 0, 2).reshape(128, 13 * 6), (1, NSB))
    # order must be (s_global, j): s_global = t*13 + chi -> col s*6 + j
    s6 = np.concatenate([S6_CLS[s % 13] for s in range(NSUBT)], axis=1)
    c["S6_big"] = s6.astype(bf16)                       # [128, 3120]
    s12 = np.concatenate(
        [np.repeat(S6_CLS[s % 13], 2, axis=1) for s in range(NSUBT)], axis=1)
    c["S12_big"] = s12.astype(bf16)                     # [128, 6240]
    s64 = np.concatenate([S64T_CLS[chi] for chi in range(13)], axis=1)
    c["S64T_all"] = np.concatenate([s64, s64], axis=0).astype(bf16)  # [128,1664]
    oh = np.zeros((3, 3 * 128), np.float32)
    for m in range(3):
        oh[m, m * 128:(m + 1) * 128] = 1.0
    c["OH3"] = oh.astype(bf16)                          # [3, 384]
    ident = np.eye(128, dtype=np.float32)
    c["IDENT"] = ident.astype(bf16)                     # [128, 128]
    c["ONES3"] = np.ones((3, 1), np.float32).astype(bf16)
    return c


CONSTS = _consts()


def _enable_ldw_opt():
    # compile_bir_kernel hardcodes --enable-ldw-opt=false; rewrite it so
    # walrus can optimize LDWEIGHTS scheduling for this bf16-only kernel.
    from concourse import bass_utils as bu
    if getattr(bu, "_ldw_patched", False):
        return
    orig = bu.run_command

    def run_command(cmd, *a, **k):
        cmd = [c.replace("--enable-ldw-opt=false", "--enable-ldw-opt=false")
               if isinstance(c, str) else c for c in cmd]
        return orig(cmd, *a, **k)

    bu.run_command = run_command
    bu._ldw_patched = True
    try:
        from concourse import bass2jax as b2j
        if getattr(b2j, "run_command", None) is orig:
            b2j.run_command = run_command
    except Exception:
        pass


def _build_bass():
    import concourse.bass as bass
    import concourse.tile as tile
    from concourse import bacc, mybir

    _enable_ldw_opt()

    f32, bft, i32 = mybir.dt.float32, mybir.dt.bfloat16, mybir.dt.int32
    AF = mybir.ActivationFunctionType
    OP = mybir.AluOpType
    AX = mybir.AxisListType

    nc = bacc.Bacc("TRN2", target_bir_lowering=False, debug=False,
                   num_devices=NCORES)

    def din(name, shape, dt):
        return nc.dram_tensor(name, shape, dt, kind="ExternalInput").ap()

    nbr = din("nbr", [E, D], bft)
    nbr_nat = din("nbr_nat", [128, NSUBT * D], bft)
    curT_f = din("curT_f", [D, NS], f32)
    curT_b = din("curT_b", [D, NS], bft)
    conn = din("conn", [128, NSUBT], i32)
    wnames = ["Wl1", "Wl2", "Wm1", "Wm2", "Wu1", "Wu2", "Wc1", "Wc2"]
    W = {k: din(k, [D, D], bft) for k in wnames}
    W["Wg1"] = din("Wg1", [D, HG], bft)
    W["Wg2"] = din("Wg2", [HG, 3], bft)
    bias_in = {
        "b_local": din("b_local", [D, 1], f32),
        "b_upd": din("b_upd", [D, 1], f32),
        "b_cnf": din("b_cnf", [D, 1], f32),
        "b_msg": din("b_msg", [D, 1], f32),
        "b_g1": din("b_g1", [HG, 1], f32),
        "b_g2": din("b_g2", [3, 1], f32),
    }
    S6_d = din("S6_big", [128, NSUBT * 6], bft)
    S12_d = din("S12_big", [128, NSUBT * 12], bft)
    S64T_d = din("S64T_all", [128, 13 * 128], bft)
    OH3_d = din("OH3", [3, 384], bft)
    ID_d = din("IDENT", [128, 128], bft)
    ONES3_d = din("ONES3", [3, 1], bft)
    outT = nc.dram_tensor("outT", [D, NS], f32, kind="ExternalOutput").ap()

    with tile.TileContext(nc) as tc, ExitStack() as ctx:
        const = ctx.enter_context(tc.tile_pool(name="const", bufs=1))
        build = ctx.enter_context(tc.tile_pool(name="build", bufs=1))
        big = ctx.enter_context(tc.tile_pool(name="big", bufs=1))
        stream = ctx.enter_context(tc.tile_pool(name="stream", bufs=2))
        temp1 = ctx.enter_context(tc.tile_pool(name="temp1", bufs=1))
        ps_long = ctx.enter_context(tc.tile_pool(name="ps_long", bufs=2,
                                                 space="PSUM"))
        ps = ctx.enter_context(tc.tile_pool(name="ps", bufs=4, space="PSUM"))

        # ---------- load constants / weights ----------
        wt = {}
        for k in wnames:
            t = const.tile([D, D], bft, tag=k)
            nc.sync.dma_start(t[:], W[k][:])
            wt[k] = t
        wg1 = const.tile([D, HG], bft)
        nc.sync.dma_start(wg1[:], W["Wg1"][:])
        wg2 = const.tile([HG, 3], bft)
        nc.sync.dma_start(wg2[:], W["Wg2"][:])
        bias = {}
        for k, ap in bias_in.items():
            t = const.tile(list(ap.shape), mybir.dt.float32, tag=k)
            nc.sync.dma_start(t[:], ap[:])
            bias[k] = t
        s6c = build.tile([128, NSUBT * 6], bft)
        nc.sync.dma_start(s6c[:], S6_d[:])
        s12c = build.tile([128, NSUBT * 12], bft)
        nc.sync.dma_start(s12c[:], S12_d[:])
        s64t = const.tile([128, 13 * 128], bft)
        nc.sync.dma_start(s64t[:], S64T_d[:])
        oh3 = const.tile([3, 384], bft)
        nc.sync.dma_start(oh3[:], OH3_d[:])
        ident = const.tile([128, 128], bft)
        nc.sync.dma_start(ident[:], ID_d[:])
        ones3 = const.tile([3, 1], bft)
        nc.sync.dma_start(ones3[:], ONES3_d[:])
        curTb = const.tile([D, NS], bft)
        nc.sync.dma_start(curTb[:], curT_b[:])
        curTf = const.tile([D, NS], mybir.dt.float32)
        nc.sync.dma_start(curTf[:], curT_f[:])
        conn_sb = const.tile([128, NSUBT], i32)
        nc.sync.dma_start(conn_sb[:], conn[:])
        zrow = const.tile([1, 128], bft)
        nc.vector.memset(zrow[:], 0.0)
        zdum = const.tile([1, CHUNK], bft)
        nc.vector.memset(zdum[:], 0.0)

        # ---------- bulk mask building ----------
        w3 = big.tile([128, NSUBT * 3], bft)         # col s*3+m, m in (l,d,f)
        for m, val in enumerate((0, 2, 1)):          # l->conn==0 d->2 f->1
            nc.vector.tensor_scalar(w3[:, m::3], conn_sb[:], val, None,
                                    OP.is_equal)
        B_ld = big.tile([128, NSUBT * 12], bft)
        w3v = w3[:].rearrange("p (s c) -> p s c", c=3)
        in1 = w3v[:, :, 0:2].unsqueeze(2).broadcast_to([128, NSUBT, 6, 2])
        nc.vector.tensor_tensor(
            B_ld[:].rearrange("p (s j c) -> p s j c", j=6, c=2),
            s12c[:].rearrange("p (s j c) -> p s j c", j=6, c=2),
            in1, OP.mult)
        B_f = big.tile([128, NSUBT * 6], bft)
        in1f = w3v[:, :, 2:3].broadcast_to([128, NSUBT, 6])
        nc.vector.tensor_tensor(
            B_f[:].rearrange("p (s j) -> p s j", j=6),
            s6c[:].rearrange("p (s j) -> p s j", j=6),
            in1f, OP.mult)

        # ---------- cpmT = Wm1.T @ curT + b_msg ;  cpm_nat per superblock ----
        cpmT = big.tile([D, NS], bft)
        for ch in range(NCHUNK):
            pm = ps.tile([128, CHUNK], mybir.dt.float32, tag="p")
            sl = slice(ch * CHUNK, (ch + 1) * CHUNK)
            nc.tensor.matmul(pm[:], wt["Wm1"][:], curTb[:, sl], start=True,
                             stop=True)
            nc.scalar.activation(cpmT[:, sl], pm[:], AF.Identity,
                                 bias=bias["b_msg"][:])
        cpm_nat = big.tile([128, NSB * 128], bft)
        for t in range(NSB):
            pt = ps.tile([64, 128], bft, tag="p")
            nc.tensor.transpose(pt[:], cpmT[:, t * 64:(t + 1) * 64], ident[:])
            nc.scalar.copy(cpm_nat[0:64, t * 128:(t + 1) * 128], pt[:])
            nc.scalar.copy(cpm_nat[64:128, t * 128:(t + 1) * 128], pt[:])

        # ---------- main superblock loop ----------
        aggldT = big.tile([128, NSB * 128], bft)   # col t*128 + 2c+m
        aggfT = big.tile([128, NSB * 64], bft)     # col t*64 + c
        def do_superblock(t):
            natT = stream.tile([128, EPB], bft, tag="natT")
            nc.sync.dma_start(natT[:], nbr[t * EPB:(t + 1) * EPB, :],
                              transpose=True)
            nat = stream.tile([128, NSUB, 128], bft, tag="nat")
            nc.sync.dma_start(
                nat[:], nbr_nat[:, t * EPB:(t + 1) * EPB].rearrange(
                    "p (s d) -> p s d", d=128))
            msgs = stream.tile([128, EPB], bft, tag="msgs")

            pagg = ps_long.tile([128, 192], mybir.dt.float32, tag="pagg")
            nc.vector.memset(pagg[:], 0.0)

            groups = [(0, 4), (4, 4), (8, 4), (12, 1)]
            for g0, gn in groups:
                pmsg = ps.tile([128, 512], mybir.dt.float32, tag="p")
                for i in range(gn):
                    s = g0 + i
                    sg = t * NSUB + s
                    csl = slice(i * 128, (i + 1) * 128)
                    nc.tensor.matmul(pmsg[:, csl],
                                     natT[:, s * 128:(s + 1) * 128],
                                     wt["Wm2"][:], start=True, stop=False)
                    half = 64 * (s % 2)
                    nc.tensor.matmul(pmsg[:, csl],
                                     s64t[half:half + 64,
                                          (s % 13) * 128:(s % 13 + 1) * 128],
                                     cpm_nat[half:half + 64,
                                             t * 128:(t + 1) * 128],
                                     start=False, stop=True)
                nc.scalar.activation(msgs[:, g0 * 128:(g0 + gn) * 128],
                                     pmsg[:, 0:gn * 128], AF.Relu)
                for i in range(gn):
                    s = g0 + i
                    sg = t * NSUB + s
                    cb2 = 2 * CB_LOC[s]
                    w = min(6, SBC - CB_LOC[s])
                    last = s == NSUB - 1
                    nc.tensor.matmul(pagg[:, cb2:cb2 + 2 * w],
                                     nat[:, s, :],
                                     B_ld[:, sg * 12:sg * 12 + 2 * w],
                                     start=False, stop=last)
                    nc.tensor.matmul(pagg[:, 128 + CB_LOC[s]:128 + CB_LOC[s] + w],
                                     msgs[:, s * 128:(s + 1) * 128],
                                     B_f[:, sg * 6:sg * 6 + w],
                                     start=False, stop=last)
            return pagg

        def evac_superblock(t, pagg):
            # evacuate aggregates with 1/cnt scaling (cell-indexed cols)
            csl = slice(t * SBC, (t + 1) * SBC)
            nc.vector.tensor_tensor(aggldT[:, t * 128:(t + 1) * 128:2],
                                    pagg[:, 0:128:2], ibc[0][:, csl], OP.mult)
            nc.vector.tensor_tensor(aggldT[:, t * 128 + 1:(t + 1) * 128:2],
                                    pagg[:, 1:128:2], ibc[1][:, csl], OP.mult)
            nc.vector.tensor_tensor(aggfT[:, t * 64:(t + 1) * 64],
                                    pagg[:, 128:192], ibc[2][:, csl], OP.mult)

        early = [do_superblock(t) for t in range(2)]

        # ---------- counts -> inv (cell layout [3, NS]) ----------
        inv_hi = big.tile([3, NS], bft)
        inv_lo = big.tile([3, NS], bft)
        for ch in range(NCHUNK):
            pc = ps.tile([3, CHUNK], mybir.dt.float32, tag="p")
            nc.vector.memset(pc[:], 0.0)
            s0 = ch * SB_PER_CHUNK * NSUB
            for sl in range(SB_PER_CHUNK * NSUB):
                s = s0 + sl
                cb = (s // NSUB) * SBC - ch * CHUNK + CB_LOC[s % NSUB]
                w = min(6, SBC - CB_LOC[s % NSUB])
                last = sl == SB_PER_CHUNK * NSUB - 1
                nc.tensor.matmul(pc[:, cb:cb + w], w3[:, 3 * s:3 * s + 3],
                                 s6c[:, 6 * s:6 * s + w], start=False,
                                 stop=last)
            csl = slice(ch * CHUNK, (ch + 1) * CHUNK)
            cnt1 = temp1.tile([3, CHUNK], mybir.dt.float32, tag="cnt1")
            nc.vector.tensor_scalar(cnt1[:], pc[:], 1.0, None, OP.max)
            invf = temp1.tile([3, CHUNK], mybir.dt.float32, tag="invf")
            nc.vector.reciprocal(invf[:], cnt1[:])
            nc.vector.tensor_copy(inv_hi[:, csl], invf[:])
            lo_t = temp1.tile([3, CHUNK], mybir.dt.float32, tag="lot")
            nc.vector.tensor_tensor(lo_t[:], invf[:], inv_hi[:, csl],
                                    OP.subtract)
            nc.vector.tensor_copy(inv_lo[:, csl], lo_t[:])

        # broadcast inv rows to 128 partitions (bf16, hi+lo): ibc[m]
        ibc = []
        for m in range(3):
            t = big.tile([128, NS], bft, tag=f"ibc{m}")
            ibc.append(t)
        for m in range(3):
            for ch in range(NCHUNK):
                pb = ps.tile([128, CHUNK], mybir.dt.float32, tag="p")
                sl = slice(ch * CHUNK, (ch + 1) * CHUNK)
                nc.tensor.matmul(pb[:], oh3[:, m * 128:(m + 1) * 128],
                                 inv_hi[:, sl], start=True, stop=False)
                mm = nc.tensor.matmul(pb[:], oh3[:, m * 128:(m + 1) * 128],
                                       inv_lo[:, sl], start=False, stop=True)
                mm.ins.ldweights = False
                nc.scalar.copy(ibc[m][:, sl], pb[:])


        for t, pg in enumerate(early):
            evac_superblock(t, pg)
        for t in range(2, NSB):
            evac_superblock(t, do_superblock(t))

        # ---------- second stage (transposed, chunked) ----------
        localT = big.tile([128, NS], bft)
        funcT = big.tile([128, NS], bft)

        def agg_view(base_off, ch):
            # aggldT cols (t*128 + 2c + m) for cells of chunk ch
            v = aggldT[:, ch * SB_PER_CHUNK * 128 + base_off:
                       (ch + 1) * SB_PER_CHUNK * 128:2]
            return v.rearrange("p (t c) -> p t c", c=64)

        for ch in range(NCHUNK):
            sl = slice(ch * CHUNK, (ch + 1) * CHUNK)
            pl = ps.tile([128, CHUNK], mybir.dt.float32, tag="p")
            nc.tensor.matmul(pl[:], wt["Wl1"][:], curTb[:, sl], start=True,
                             stop=False)
            nc.tensor.matmul(
                pl[:].rearrange("p (t c) -> p t c", c=64),
                wt["Wl2"][:], agg_view(0, ch), start=False, stop=True)
            nc.scalar.activation(localT[:, sl], pl[:], AF.Tanh,
                                 bias=bias["b_local"][:])
            pf = ps.tile([128, CHUNK], mybir.dt.float32, tag="p")
            nc.tensor.matmul(pf[:], wt["Wu1"][:], curTb[:, sl], start=True,
                             stop=False)
            nc.tensor.matmul(
                pf[:].rearrange("p (t c) -> p t c", c=64),
                wt["Wu2"][:],
                aggfT[:, ch * SB_PER_CHUNK * 64:(ch + 1) * SB_PER_CHUNK * 64]
                .rearrange("p (t c) -> p t c", c=64),
                start=False, stop=True)
            nc.scalar.activation(funcT[:, sl], pf[:], AF.Tanh,
                                 bias=bias["b_upd"][:])

        # CNF: 3 Euler steps
        s_prev = curTf
        s_prev_bf = curTb
        for step in range(CNF_STEPS):
            s_next = big.tile([128, NS], mybir.dt.float32, tag=f"s{step % 2}")
            for ch in range(NCHUNK):
                sl = slice(ch * CHUNK, (ch + 1) * CHUNK)
                pp = ps.tile([128, CHUNK], mybir.dt.float32, tag="p")
                nc.tensor.matmul(pp[:], wt["Wc1"][:], s_prev_bf[:, sl],
                                 start=True, stop=False)
                nc.tensor.matmul(
                    pp[:].rearrange("p (t c) -> p t c", c=64),
                    wt["Wc2"][:], agg_view(1, ch), start=False, stop=True)
                th = temp1.tile([128, CHUNK], mybir.dt.float32, tag="th")
                nc.scalar.activation(th[:], pp[:], AF.Tanh,
                                     bias=bias["b_cnf"][:])
                nc.vector.tensor_scalar(th[:], th[:], DTC, None, OP.mult)
                nc.vector.tensor_tensor(s_next[:, sl], s_prev[:, sl], th[:],
                                        OP.add)
            s_prev = s_next
            if step < CNF_STEPS - 1:
                nb = big.tile([128, NS], bft, tag="sbf")
                nc.vector.tensor_copy(nb[:], s_next[:])
                s_prev_bf = nb

        # gating + final mix, per chunk
        for ch in range(NCHUNK):
            sl = slice(ch * CHUNK, (ch + 1) * CHUNK)
            ph = ps.tile([HG, CHUNK], mybir.dt.float32, tag="p")
            nc.tensor.matmul(ph[:], wg1[:], curTb[:, sl], start=True,
                             stop=True)
            hT = temp1.tile([HG, CHUNK], bft, tag="hT")
            nc.scalar.activation(hT[:], ph[:], AF.Relu, bias=bias["b_g1"][:])
            pz = ps.tile([3, CHUNK], mybir.dt.float32, tag="p")
            nc.tensor.matmul(pz[:], wg2[:], hT[:], start=True, stop=True)
            e3 = temp1.tile([3, CHUNK], mybir.dt.float32, tag="e3")
            nc.scalar.activation(e3[:], pz[:], AF.Exp, bias=bias["b_g2"][:])
            e_hi = temp1.tile([3, CHUNK], bft, tag="ehi")
            nc.vector.tensor_copy(e_hi[:], e3[:])
            e_lof = temp1.tile([3, CHUNK], mybir.dt.float32, tag="elof")
            nc.vector.tensor_tensor(e_lof[:], e3[:], e_hi[:], OP.subtract)
            e_lo = temp1.tile([3, CHUNK], bft, tag="elo")
            nc.vector.tensor_copy(e_lo[:], e_lof[:])
            psum1 = ps.tile([1, CHUNK], mybir.dt.float32, tag="p")
            nc.tensor.matmul(psum1[:], ones3[:], e_hi[:], start=True,
                             stop=False)
            mm = nc.tensor.matmul(psum1[:], ones3[:], e_lo[:], start=False,
                                  stop=True)
            mm.ins.ldweights = False
            rec = temp1.tile([1, CHUNK], mybir.dt.float32, tag="rec")
            nc.vector.reciprocal(rec[:], psum1[:])
            rbc = temp1.tile([128, CHUNK], mybir.dt.float32, tag="rbc")
            nc.gpsimd.partition_broadcast(rbc[:], rec[:])

            pe = []
            for m in range(3):
                p = ps.tile([128, CHUNK], mybir.dt.float32, tag="p")
                nc.tensor.matmul(p[:], oh3[:, m * 128:(m + 1) * 128],
                                 e_hi[:], start=True, stop=False)
                mm = nc.tensor.matmul(p[:], oh3[:, m * 128:(m + 1) * 128],
                                       e_lo[:], start=False, stop=True)
                mm.ins.ldweights = False
                pe.append(p)
            acc = temp1.tile([128, CHUNK], mybir.dt.float32, tag="acc")
            tmp = temp1.tile([128, CHUNK], mybir.dt.float32, tag="tmp")
            nc.vector.tensor_tensor(acc[:], localT[:, sl], pe[0][:], OP.mult)
            nc.vector.tensor_tensor(tmp[:], funcT[:, sl], pe[1][:], OP.mult)
            nc.vector.tensor_tensor(acc[:], acc[:], tmp[:], OP.add)
            nc.vector.tensor_tensor(tmp[:], s_prev[:, sl], pe[2][:], OP.mult)
            nc.vector.tensor_tensor(acc[:], acc[:], tmp[:], OP.add)
            nc.vector.tensor_tensor(acc[:], acc[:], rbc[:], OP.mult)
            nc.sync.dma_start(outT[:, sl], acc[:])

    nc.compile()
    return nc


_NC_CACHE = None


def _get_nc():
    global _NC_CACHE
    if _NC_CACHE is None:
        _NC_CACHE = _build_bass()
    return _NC_CACHE


def _prep_core_inputs(cur, nbr, conn, weights):
    """cur [NS, D] f32, nbr [NS, K, D] f32, conn [NS, K] i32 -> input map."""
    m = {}
    nf = nbr.reshape(E, D).astype(bf16)
    m["nbr"] = nf
    m["nbr_nat"] = np.ascontiguousarray(
        nf.reshape(NSUBT, 128, D).transpose(1, 0, 2)).reshape(128, NSUBT * D)
    ct = np.ascontiguousarray(cur.T)
    m["curT_f"] = ct.astype(np.float32)
    m["curT_b"] = ct.astype(bf16)
    m["conn"] = np.ascontiguousarray(
        conn.reshape(NSUBT, 128).T).astype(np.int32)
    Wl, Wm, Wu, Wc = (weights["W_local"], weights["W_msg"],
                      weights["W_upd"], weights["W_cnf"])
    m["Wl1"], m["Wl2"] = Wl[:D].astype(bf16), Wl[D:].astype(bf16)
    m["Wm1"], m["Wm2"] = Wm[:D].astype(bf16), Wm[D:].astype(bf16)
    m["Wu1"], m["Wu2"] = Wu[:D].astype(bf16), Wu[D:].astype(bf16)
    m["Wc1"], m["Wc2"] = Wc[:D].astype(bf16), Wc[D:].astype(bf16)
    m["Wg1"] = weights["W_g1"].astype(bf16)
    m["Wg2"] = weights["W_g2"].astype(bf16)
    m["b_local"] = weights["b_local"].reshape(D, 1).astype(np.float32)
    m["b_upd"] = weights["b_upd"].reshape(D, 1).astype(np.float32)
    m["b_cnf"] = weights["b_cnf"].reshape(D, 1).astype(np.float32)
    m["b_msg"] = weights["b_msg"].reshape(D, 1).astype(np.float32)
    m["b_g1"] = weights["b_g1"].reshape(HG, 1).astype(np.float32)
    m["b_g2"] = weights["b_g2"].reshape(3, 1).astype(np.float32)
    for k, v in CONSTS.items():
        m[k] = v
    return m


def kernel(**inputs):
    from concourse.bass_utils import run_bass_kernel_spmd

    cur = np.asarray(inputs["current_state"], np.float32)
    nbr = np.asarray(inputs["neighbor_states"], np.float32)
    conn = np.asarray(inputs["conn_type"], np.int32)
    weights = {k: np.asarray(v, np.float32) for k, v in inputs.items()
               if k not in ("current_state", "neighbor_states", "conn_type")}

    npad = NCORES * NS
    cur_p = np.zeros((npad, D), np.float32)
    cur_p[:N_CELLS] = cur
    nbr_p = np.zeros((npad, K, D), np.float32)
    nbr_p[:N_CELLS] = nbr
    conn_p = np.full((npad, K), 3, np.int32)
    conn_p[:N_CELLS] = conn

    in_maps = []
    for c in range(NCORES):
        sl = slice(c * NS, (c + 1) * NS)
        in_maps.append(_prep_core_inputs(cur_p[sl], nbr_p[sl], conn_p[sl],
                                         weights))
    nc = _get_nc()
    res = run_bass_kernel_spmd(nc, in_maps, list(range(NCORES)))
    out = np.concatenate([res.results[c]["outT"].T for c in range(NCORES)],
                         axis=0)
    return np.ascontiguousarray(out[:N_CELLS]).astype(np.float32)


if __name__ == "__main__":
    pass



# revision 13
# speedup vs baseline: 1.7849x; 1.7849x over previous
"""Trainium2 Bass kernel for nn_MoEConnectionProcessor.

Data-parallel over cells: 8 cores x 2560 padded cells (19683 real).

v2 layout strategy (transposed messages):
  - nbr shipped twice from host: natT [d, edge] (moving operand for all
    per-edge projections; Wm2 stays stationary across long streams) and
    nat [edge, d] subtile-major (stationary for the masked l/d
    aggregation matmuls).
  - messages live transposed [dout, edge]: proj = Wm2^T @ natT chunk,
    plus one accumulate matmul whose stationary is [cpm_nat | ones] and
    whose moving operand is a host-built masked staircase SELC carrying
    m_f selectors (rows 0-31) and a -32768 penalty row, so relu both
    applies the cur-projection broadcast and zeroes masked edges.
  - functional aggregation = DVE segmented reduce over 26-edge groups.
  - l/d aggregation masks ship pre-scaled by 1/count (bf16 hi+lo), so
    PSUM holds final normalized aggregates and evacuation is a copy.
  - gating uses reciprocal_approx_fast + gpsimd partition broadcasts.
"""

import numpy as np
import ml_dtypes
from contextlib import ExitStack

N_CELLS, K, D, HG = 19683, 26, 128, 64
NCORES = 8
NS = 2560                 # padded cells per core
E = NS * K                # 66560 edges per core
SBC = 64                  # cells per superblock (l/d agg granularity)
NSB = NS // SBC           # 40 superblocks
NSUB = 13                 # subtiles (128 edges) per superblock
HCELL = 32                # cells per half-superblock (msg granularity)
EPH = HCELL * K           # 832 edges per half-superblock
NHB = NS // HCELL         # 80 half-superblocks
NSUBT = NS * K // 128     # 520 subtiles per core
CHUNK = 512
NCHUNK = NS // CHUNK      # 5
CNF_STEPS, DTC = 3, 0.1
PEN = -32768.0

bf16 = ml_dtypes.bfloat16


def _cb_loc():
    # first local cell of subtile chi within its superblock
    return [(chi * 128) // K for chi in range(NSUB)]


CB_LOC = _cb_loc()


def _consts():
    c = {}
    ident = np.eye(128, dtype=np.float32)
    c["IDENT"] = ident.astype(bf16)                     # [128, 128]
    c["ONES2"] = np.ones((2, 128), np.float32).astype(bf16)
    c["ONES3"] = np.ones((3, 1), np.float32)
    return c


CONSTS = _consts()


def _build_bass():
    import concourse.bass as bass
    import concourse.tile as tile
    from concourse import bacc, mybir

    f32, bft, i32 = mybir.dt.float32, mybir.dt.bfloat16, mybir.dt.int32
    AF = mybir.ActivationFunctionType
    OP = mybir.AluOpType
    AX = mybir.AxisListType

    nc = bacc.Bacc("TRN2", target_bir_lowering=False, debug=False,
                   num_devices=NCORES)

    def din(name, shape, dt):
        return nc.dram_tensor(name, shape, dt, kind="ExternalInput").ap()

    natT_d = din("natT", [128, E], bft)
    nat_d = din("nat", [128, NSUBT * D], bft)
    selc_d = din("SELC", [33, E], bft)
    bhi_d = din("B_hi", [128, NSUBT * 12], bft)
    blo_d = din("B_lo", [128, NSUBT * 12], bft)
    invf_d = din("INVF2", [2, NS], bft)
    curT_f = din("curT_f", [D, NS], f32)
    curT_b = din("curT_b", [D, NS], bft)
    wnames = ["Wl1", "Wl2", "Wm1", "Wm2", "Wu1", "Wu2", "Wc1", "Wc2"]
    W = {k: din(k, [D, D], bft) for k in wnames}
    W["Wg1"] = din("Wg1", [D, HG], bft)
    W["Wg2"] = din("Wg2", [HG, 3], bft)
    bias_in = {
        "b_local": din("b_local", [D, 1], f32),
        "b_upd": din("b_upd", [D, 1], f32),
        "b_cnf": din("b_cnf", [D, 1], f32),
        "b_msg": din("b_msg", [D, 1], f32),
        "b_g1": din("b_g1", [HG, 1], f32),
        "b_g2": din("b_g2", [3, 1], f32),
    }
    ID_d = din("IDENT", [128, 128], bft)
    ONES2_d = din("ONES2", [2, 128], bft)
    ONES3_d = din("ONES3", [3, 1], f32)
    outT = nc.dram_tensor("outT", [D, NS], f32, kind="ExternalOutput").ap()

    with tile.TileContext(nc) as tc, ExitStack() as ctx:
        const = ctx.enter_context(tc.tile_pool(name="const", bufs=1))
        big = ctx.enter_context(tc.tile_pool(name="big", bufs=1))
        st_natT = ctx.enter_context(tc.tile_pool(name="st_natT", bufs=3))
        st_selc = ctx.enter_context(tc.tile_pool(name="st_selc", bufs=3))
        st_nat = ctx.enter_context(tc.tile_pool(name="st_nat", bufs=2))
        st_msgs = ctx.enter_context(tc.tile_pool(name="st_msgs", bufs=2))
        temp1 = ctx.enter_context(tc.tile_pool(name="temp1", bufs=2))
        psM = ctx.enter_context(tc.tile_pool(name="psM", bufs=2,
                                             space="PSUM"))
        psG = ctx.enter_context(tc.tile_pool(name="psG", bufs=2,
                                             space="PSUM"))
        psC = ctx.enter_context(tc.tile_pool(name="psC", bufs=2,
                                             space="PSUM"))

        # ---------- load constants / weights ----------
        wt = {}
        for k in wnames:
            t = const.tile([D, D], bft, tag=k, name=k)
            nc.sync.dma_start(t[:], W[k][:])
            wt[k] = t
        wg1 = const.tile([D, HG], bft)
        nc.sync.dma_start(wg1[:], W["Wg1"][:])
        wg2 = const.tile([HG, 3], bft)
        nc.sync.dma_start(wg2[:], W["Wg2"][:])
        bias = {}
        for k, ap in bias_in.items():
            t = const.tile(list(ap.shape), f32, tag=k, name=k)
            nc.sync.dma_start(t[:], ap[:])
            bias[k] = t
        ident = const.tile([128, 128], bft)
        nc.sync.dma_start(ident[:], ID_d[:])
        ones2 = const.tile([2, 128], bft)
        nc.sync.dma_start(ones2[:], ONES2_d[:])
        ones3 = const.tile([3, 1], f32)
        nc.sync.dma_start(ones3[:], ONES3_d[:])
        curTb = const.tile([D, NS], bft)
        nc.sync.dma_start(curTb[:], curT_b[:])
        curTf = const.tile([D, NS], f32)
        nc.sync.dma_start(curTf[:], curT_f[:])
        bhi = const.tile([128, NSUBT * 12], bft)
        nc.sync.dma_start(bhi[:], bhi_d[:])
        blo = const.tile([128, NSUBT * 12], bft)
        nc.sync.dma_start(blo[:], blo_d[:])
        invf2 = const.tile([2, NS], bft)
        nc.sync.dma_start(invf2[:], invf_d[:])

        # ---------- cpmT = Wm1.T @ curT + b_msg ----------
        cpmT = big.tile([D, NS], bft)
        for ch in range(NCHUNK):
            pm = psC.tile([128, CHUNK], f32, tag="p")
            sl = slice(ch * CHUNK, (ch + 1) * CHUNK)
            mm = nc.tensor.matmul(pm[:], wt["Wm1"][:], curTb[:, sl],
                                  start=True, stop=True)
            if ch > 0:
                mm.ins.ldweights = False
            nc.scalar.activation(cpmT[:, sl], pm[:], AF.Identity,
                                 bias=bias["b_msg"][:])

        # cpm_natA [33, 80*128]: rows 0-31 cell-major cpm per half-sb,
        # row 32 = ones (for the SELC penalty row)
        cpm_natA = big.tile([33, NHB * 128], bft)
        nc.vector.memset(cpm_natA[32:33, :], 1.0)
        for h2 in range(0, NHB, 4):
            pt = psC.tile([32, 512], bft, tag="p")
            for i in range(4):
                h = h2 + i
                nc.tensor.transpose(pt[:, i * 128:(i + 1) * 128],
                                    cpmT[:, h * 32:(h + 1) * 32], ident[:])
            nc.scalar.copy(
                cpm_natA[0:32, h2 * 128:(h2 + 4) * 128], pt[:])

        # ---------- main loop: half-superblocks ----------
        aggF32 = big.tile([128, NS], f32)     # functional agg (unnormalized)
        aggldT = big.tile([128, NSB * 128], bft)  # col t*128 + 2c+m (l,d)

        def do_half(h):
            natT_h = st_natT.tile([128, EPH], bft, tag="natT")
            nc.sync.dma_start(natT_h[:], natT_d[:, h * EPH:(h + 1) * EPH])
            selc_h = st_selc.tile([33, EPH], bft, tag="selc")
            nc.sync.dma_start(selc_h[:], selc_d[:, h * EPH:(h + 1) * EPH])

            # [128, 1024] so the slot is exactly 2 PSUM banks (bank-aligned)
            pmsg = psM.tile([128, 1024], f32, tag="pm")
            mm = nc.tensor.matmul(pmsg[:, 0:512], wt["Wm2"][:],
                                  natT_h[:, 0:512], start=True, stop=False)
            mm2 = nc.tensor.matmul(pmsg[:, 512:EPH], wt["Wm2"][:],
                                   natT_h[:, 512:EPH], start=True, stop=False)
            mm2.ins.ldweights = False
            stat = cpm_natA[:, h * 128:(h + 1) * 128]
            mm3 = nc.tensor.matmul(pmsg[:, 0:512], stat, selc_h[:, 0:512],
                                   start=False, stop=True)
            mm4 = nc.tensor.matmul(pmsg[:, 512:EPH], stat,
                                   selc_h[:, 512:EPH], start=False, stop=True)
            mm4.ins.ldweights = False

            msgs = st_msgs.tile([128, EPH], bft, tag="msgs")
            nc.scalar.activation(msgs[:], pmsg[:, 0:EPH], AF.Relu)
            nc.vector.tensor_reduce(
                aggF32[:, h * HCELL:(h + 1) * HCELL],
                msgs[:].rearrange("p (c k) -> p c k", k=K),
                AX.X, OP.add)

        def do_ld(t):
            nat_t = st_nat.tile([128, NSUB * 128], bft, tag="nat")
            nc.sync.dma_start(
                nat_t[:], nat_d[:, t * NSUB * 128:(t + 1) * NSUB * 128])
            # full-bank slot ([128, 512] f32 = 1 bank); only 128 cols used
            pagg_t = psG.tile([128, 512], f32, tag="pg")
            pagg = pagg_t[:, 0:128]
            for s in range(NSUB):
                sg = t * NSUB + s
                cb2 = 2 * CB_LOC[s]
                w = min(6, SBC - CB_LOC[s])
                nat_s = nat_t[:, s * 128:(s + 1) * 128]
                nc.tensor.matmul(pagg[:, cb2:cb2 + 2 * w], nat_s,
                                 bhi[:, sg * 12:sg * 12 + 2 * w],
                                 start=(s == 0), stop=False)
                mm = nc.tensor.matmul(pagg[:, cb2:cb2 + 2 * w], nat_s,
                                      blo[:, sg * 12:sg * 12 + 2 * w],
                                      start=False, stop=(s == NSUB - 1))
                mm.ins.ldweights = False
            nc.vector.tensor_copy(aggldT[:, t * 128:(t + 1) * 128], pagg[:])

        for t in range(NSB):
            do_ld(t)
            do_half(2 * t)
            do_half(2 * t + 1)

        # ---------- normalize functional aggregate ----------
        aggFb = big.tile([128, NS], bft)
        for ch in range(NCHUNK):
            sl = slice(ch * CHUNK, (ch + 1) * CHUNK)
            pb = psC.tile([128, CHUNK], f32, tag="p")
            mm = nc.tensor.matmul(pb[:], ones2[:], invf2[:, sl], start=True,
                                  stop=True)
            if ch > 0:
                mm.ins.ldweights = False
            nc.vector.tensor_tensor(aggFb[:, sl], aggF32[:, sl], pb[:],
                                    OP.mult)

        # ---------- second stage (transposed, chunked) ----------
        localT = big.tile([128, NS], bft)
        funcT = big.tile([128, NS], bft)

        def agg_view(base_off, ch):
            # aggldT cols (t*128 + 2c + m) for cells of chunk ch
            v = aggldT[:, ch * 8 * 128 + base_off:(ch + 1) * 8 * 128:2]
            return v.rearrange("p (t c) -> p t c", c=64)

        for ch in range(NCHUNK):
            sl = slice(ch * CHUNK, (ch + 1) * CHUNK)
            pl = psC.tile([128, CHUNK], f32, tag="p")
            nc.tensor.matmul(pl[:], wt["Wl1"][:], curTb[:, sl], start=True,
                             stop=False)
            nc.tensor.matmul(
                pl[:].rearrange("p (t c) -> p t c", c=64),
                wt["Wl2"][:], agg_view(0, ch), start=False, stop=True)
            nc.scalar.activation(localT[:, sl], pl[:], AF.Tanh,
                                 bias=bias["b_local"][:])
            pf = psC.tile([128, CHUNK], f32, tag="p")
            nc.tensor.matmul(pf[:], wt["Wu1"][:], curTb[:, sl], start=True,
                             stop=False)
            nc.tensor.matmul(pf[:], wt["Wu2"][:], aggFb[:, sl],
                             start=False, stop=True)
            nc.scalar.activation(funcT[:, sl], pf[:], AF.Tanh,
                                 bias=bias["b_upd"][:])

        # CNF: 3 Euler steps
        s_prev = curTf
        s_prev_bf = curTb
        for step in range(CNF_STEPS):
            s_next = big.tile([128, NS], f32, tag=f"s{step % 2}",
                              name=f"s_next{step}")
            for ch in range(NCHUNK):
                sl = slice(ch * CHUNK, (ch + 1) * CHUNK)
                pp = psC.tile([128, CHUNK], f32, tag="p")
                nc.tensor.matmul(pp[:], wt["Wc1"][:], s_prev_bf[:, sl],
                                 start=True, stop=False)
                nc.tensor.matmul(
                    pp[:].rearrange("p (t c) -> p t c", c=64),
                    wt["Wc2"][:], agg_view(1, ch), start=False, stop=True)
                th = temp1.tile([128, CHUNK], f32, tag="th")
                nc.scalar.activation(th[:], pp[:], AF.Tanh,
                                     bias=bias["b_cnf"][:])
                nc.vector.scalar_tensor_tensor(
                    s_next[:, sl], th[:], DTC, s_prev[:, sl],
                    OP.mult, OP.add)
            s_prev = s_next
            if step < CNF_STEPS - 1:
                nb = big.tile([128, NS], bft, tag="sbf", name=f"sbf{step}")
                nc.vector.tensor_copy(nb[:], s_next[:])
                s_prev_bf = nb

        # ---------- gating + final mix, fused per chunk ----------
        for ch in range(NCHUNK):
            sl = slice(ch * CHUNK, (ch + 1) * CHUNK)
            ph = psC.tile([HG, CHUNK], f32, tag="p")
            nc.tensor.matmul(ph[:], wg1[:], curTb[:, sl], start=True,
                             stop=True)
            hT = temp1.tile([HG, CHUNK], bft, tag="hT")
            nc.scalar.activation(hT[:], ph[:], AF.Relu, bias=bias["b_g1"][:])
            pz = psC.tile([3, CHUNK], f32, tag="p")
            nc.tensor.matmul(pz[:], wg2[:], hT[:], start=True, stop=True)
            e3 = temp1.tile([3, CHUNK], f32, tag="e3")
            nc.scalar.activation(e3[:], pz[:], AF.Exp, bias=bias["b_g2"][:])
            p1_t = psG.tile([128, 512], f32, tag="pg")
            p1 = p1_t[0:1, 0:CHUNK]
            nc.tensor.matmul(p1[:], ones3[:], e3[:], start=True, stop=True)
            rec = temp1.tile([1, CHUNK], f32, tag="rec")
            nc.vector.reciprocal_approx_fast(rec[:], p1[:])
            rbc = temp1.tile([128, CHUNK], f32, tag="rbc")
            nc.gpsimd.partition_broadcast(rbc[:], rec[:])
            ge = []
            for m in range(3):
                # partition_broadcast needs its input on partition 0; DMA the
                # gate row down from partition m first
                erow = temp1.tile([1, CHUNK], f32, tag=f"erow{m}",
                                  name=f"erow{m}")
                nc.sync.dma_start(erow[:], e3[m:m + 1, :])
                g = temp1.tile([128, CHUNK], f32, tag=f"ge{m}",
                               name=f"ge{m}")
                nc.gpsimd.partition_broadcast(g[:], erow[:])
                ge.append(g)
            acc = temp1.tile([128, CHUNK], f32, tag="acc")
            tmp = temp1.tile([128, CHUNK], f32, tag="tmp")
            nc.vector.tensor_tensor(acc[:], localT[:, sl], ge[0][:], OP.mult)
            nc.vector.tensor_tensor(tmp[:], funcT[:, sl], ge[1][:], OP.mult)
            nc.vector.tensor_tensor(acc[:], acc[:], tmp[:], OP.add)
            nc.vector.tensor_tensor(tmp[:], s_prev[:, sl], ge[2][:], OP.mult)
            nc.vector.tensor_tensor(acc[:], acc[:], tmp[:], OP.add)
            nc.vector.tensor_tensor(acc[:], acc[:], rbc[:], OP.mult)
            nc.sync.dma_start(outT[:, sl], acc[:])

    nc.compile()
    return nc


_NC_CACHE = None


def _get_nc():
    global _NC_CACHE
    if _NC_CACHE is None:
        _NC_CACHE = _build_bass()
    return _NC_CACHE


def _split_hilo(w):
    hi = w.astype(bf16)
    lo = (w - hi.astype(np.float32)).astype(bf16)
    return hi, lo


def _prep_core_inputs(cur, nbr, conn, weights):
    """cur [NS, D] f32, nbr [NS, K, D] f32, conn [NS, K] i32 -> input map."""
    m = {}
    x = nbr.reshape(E, D).astype(bf16)
    m["natT"] = np.ascontiguousarray(x.T)                       # [128, E]
    m["nat"] = np.ascontiguousarray(
        x.reshape(NSUBT, 128, D).transpose(1, 0, 2)).reshape(128, NSUBT * D)

    cf = conn.reshape(E)
    mf = (cf == 1).astype(np.float32)
    ml = (cf == 0).astype(np.float32)
    md = (cf == 2).astype(np.float32)

    # SELC: masked staircase + penalty row
    selc = np.zeros((33, E), bf16)
    eidx = np.arange(E)
    cl32 = (eidx // K) % HCELL
    selc[cl32, eidx] = mf.astype(bf16)
    selc[32, :] = (PEN * (1.0 - mf)).astype(bf16)
    m["SELC"] = selc

    # per-cell inverse counts
    cnt_l = ml.reshape(NS, K).sum(1)
    cnt_f = mf.reshape(NS, K).sum(1)
    cnt_d = md.reshape(NS, K).sum(1)
    inv_l = 1.0 / np.maximum(cnt_l, 1.0)
    inv_f = 1.0 / np.maximum(cnt_f, 1.0)
    inv_d = 1.0 / np.maximum(cnt_d, 1.0)

    # B_hi/B_lo: staircase * mask * inv, interleaved (l,d) per cell
    cell = eidx // K          # global cell per edge
    cl64 = cell % SBC         # local cell within superblock
    s_of_e = eidx // 128      # subtile
    j = cl64 - np.asarray(CB_LOC)[s_of_e % NSUB]   # 0..5
    p_of_e = eidx % 128
    w_l = ml * inv_l[cell]
    w_d = md * inv_d[cell]
    B = np.zeros((128, NSUBT * 12), np.float32)
    B[p_of_e, s_of_e * 12 + 2 * j] = w_l
    B[p_of_e, s_of_e * 12 + 2 * j + 1] = w_d
    hi, lo = _split_hilo(B)
    m["B_hi"], m["B_lo"] = hi, lo

    ihi, ilo = _split_hilo(inv_f.astype(np.float32))
    m["INVF2"] = np.stack([ihi, ilo], axis=0)                   # [2, NS]

    ct = np.ascontiguousarray(cur.T)
    m["curT_f"] = ct.astype(np.float32)
    m["curT_b"] = ct.astype(bf16)

    Wl, Wm, Wu, Wc = (weights["W_local"], weights["W_msg"],
                      weights["W_upd"], weights["W_cnf"])
    m["Wl1"], m["Wl2"] = Wl[:D].astype(bf16), Wl[D:].astype(bf16)
    m["Wm1"], m["Wm2"] = Wm[:D].astype(bf16), Wm[D:].astype(bf16)
    m["Wu1"], m["Wu2"] = Wu[:D].astype(bf16), Wu[D:].astype(bf16)
    m["Wc1"], m["Wc2"] = Wc[:D].astype(bf16), Wc[D:].astype(bf16)
    m["Wg1"] = weights["W_g1"].astype(bf16)
    m["Wg2"] = weights["W_g2"].astype(bf16)
    m["b_local"] = weights["b_local"].reshape(D, 1).astype(np.float32)
    m["b_upd"] = weights["b_upd"].reshape(D, 1).astype(np.float32)
    m["b_cnf"] = weights["b_cnf"].reshape(D, 1).astype(np.float32)
    m["b_msg"] = weights["b_msg"].reshape(D, 1).astype(np.float32)
    m["b_g1"] = weights["b_g1"].reshape(HG, 1).astype(np.float32)
    m["b_g2"] = weights["b_g2"].reshape(3, 1).astype(np.float32)
    for k, v in CONSTS.items():
        m[k] = v
    return m


def kernel(**inputs):
    from concourse.bass_utils import run_bass_kernel_spmd

    cur = np.asarray(inputs["current_state"], np.float32)
    nbr = np.asarray(inputs["neighbor_states"], np.float32)
    conn = np.asarray(inputs["conn_type"], np.int32)
    weights = {k: np.asarray(v, np.float32) for k, v in inputs.items()
               if k not in ("current_state", "neighbor_states", "conn_type")}

    npad = NCORES * NS
    cur_p = np.zeros((npad, D), np.float32)
    cur_p[:N_CELLS] = cur
    nbr_p = np.zeros((npad, K, D), np.float32)
    nbr_p[:N_CELLS] = nbr
    conn_p = np.full((npad, K), 3, np.int32)
    conn_p[:N_CELLS] = conn

    in_maps = []
    for c in range(NCORES):
        sl = slice(c * NS, (c + 1) * NS)
        in_maps.append(_prep_core_inputs(cur_p[sl], nbr_p[sl], conn_p[sl],
                                         weights))
    nc = _get_nc()
    res = run_bass_kernel_spmd(nc, in_maps, list(range(NCORES)))
    out = np.concatenate([res.results[c]["outT"].T for c in range(NCORES)],
                         axis=0)
    return np.ascontiguousarray(out[:N_CELLS]).astype(np.float32)


if __name__ == "__main__":
    pass


# revision 25
# speedup vs baseline: 1.8504x; 1.0367x over previous
"""Trainium2 Bass kernel for nn_MoEConnectionProcessor.

Data-parallel over cells: 8 cores x 2560 padded cells (19683 real).

v2 layout strategy (transposed messages):
  - nbr shipped twice from host: natT [d, edge] (moving operand for all
    per-edge projections; Wm2 stays stationary across long streams) and
    nat [edge, d] subtile-major (stationary for the masked l/d
    aggregation matmuls).
  - messages live transposed [dout, edge]: proj = Wm2^T @ natT chunk,
    plus one accumulate matmul whose stationary is [cpm_nat | ones] and
    whose moving operand is a host-built masked staircase SELC carrying
    m_f selectors (rows 0-31) and a -32768 penalty row, so relu both
    applies the cur-projection broadcast and zeroes masked edges.
  - functional aggregation = DVE segmented reduce over 26-edge groups.
  - l/d aggregation masks ship pre-scaled by 1/count (bf16 hi+lo), so
    PSUM holds final normalized aggregates and evacuation is a copy.
  - gating uses reciprocal_approx_fast + gpsimd partition broadcasts.
"""

import numpy as np
import ml_dtypes
from contextlib import ExitStack

N_CELLS, K, D, HG = 19683, 26, 128, 64
NCORES = 8
NS = 2560                 # padded cells per core
E = NS * K                # 66560 edges per core
SBC = 64                  # cells per superblock (l/d agg granularity)
NSB = NS // SBC           # 40 superblocks
NSUB = 13                 # subtiles (128 edges) per superblock
HCELL = 32                # cells per half-superblock (msg granularity)
EPH = HCELL * K           # 832 edges per half-superblock
NHB = NS // HCELL         # 80 half-superblocks
NSUBT = NS * K // 128     # 520 subtiles per core
CHUNK = 512
NCHUNK = NS // CHUNK      # 5
CNF_STEPS, DTC = 3, 0.1
PEN = -32768.0

bf16 = ml_dtypes.bfloat16


def _cb_loc():
    # first local cell of subtile chi within its superblock
    return [(chi * 128) // K for chi in range(NSUB)]


CB_LOC = _cb_loc()


def _consts():
    c = {}
    ident = np.eye(128, dtype=np.float32)
    c["IDENT"] = ident.astype(bf16)                     # [128, 128]
    c["ONES2"] = np.ones((2, 128), np.float32).astype(bf16)
    c["ONES3"] = np.ones((3, 1), np.float32)
    return c


CONSTS = _consts()


def _enable_ldw_opt():
    # compile_bir_kernel hardcodes --enable-ldw-opt=false; rewrite it so
    # walrus schedules LDWEIGHTS into the PE background weight buffer.
    from concourse import bass_utils as bu
    if getattr(bu, "_ldw_patched", False):
        return
    orig = bu.run_command

    def run_command(cmd, *a, **k):
        # walrus --enable-ldw-opt=true rejects bacc's pre-split standalone
        # InstLdweights, so the flag must stay false; keep the hook for
        # future command rewrites.
        return orig(cmd, *a, **k)

    bu.run_command = run_command
    bu._ldw_patched = True
    try:
        from concourse import bass2jax as b2j
        if getattr(b2j, "run_command", None) is orig:
            b2j.run_command = run_command
    except Exception:
        pass


def _build_bass():
    import concourse.bass as bass
    import concourse.tile as tile
    from concourse import bacc, mybir

    _enable_ldw_opt()

    f32, bft, i32 = mybir.dt.float32, mybir.dt.bfloat16, mybir.dt.int32
    f8e5 = mybir.dt.float8e5
    AF = mybir.ActivationFunctionType
    OP = mybir.AluOpType
    AX = mybir.AxisListType

    nc = bacc.Bacc("TRN2", target_bir_lowering=False, debug=False,
                   num_devices=NCORES)

    def din(name, shape, dt):
        return nc.dram_tensor(name, shape, dt, kind="ExternalInput").ap()

    natT_d = din("natT", [128, E], bft)
    nat_d = din("nat", [128, NSUBT * D], bft)
    selc_d = din("SELC", [33, E], f8e5)
    bhi_d = din("B_hi", [128, NSUBT * 12], bft)
    invf_d = din("INVF2", [2, NS], bft)
    curT_f = din("curT_f", [D, NS], f32)
    curT_b = din("curT_b", [D, NS], bft)
    wnames = ["Wl1", "Wl2", "Wm1", "Wm2", "Wu1", "Wu2", "Wc1", "Wc2"]
    W = {k: din(k, [D, D], bft) for k in wnames}
    W["Wg1"] = din("Wg1", [D, HG], bft)
    W["Wg2"] = din("Wg2", [HG, 3], bft)
    bias_in = {
        "b_local": din("b_local", [D, 1], f32),
        "b_upd": din("b_upd", [D, 1], f32),
        "b_cnf": din("b_cnf", [D, 1], f32),
        "b_msg": din("b_msg", [D, 1], f32),
        "b_g1": din("b_g1", [HG, 1], f32),
        "b_g2": din("b_g2", [3, 1], f32),
    }
    ID_d = din("IDENT", [128, 128], bft)
    ONES2_d = din("ONES2", [2, 128], bft)
    ONES3_d = din("ONES3", [3, 1], f32)
    outT = nc.dram_tensor("outT", [D, NS], bft, kind="ExternalOutput").ap()

    with tile.TileContext(nc) as tc, ExitStack() as ctx:
        const = ctx.enter_context(tc.tile_pool(name="const", bufs=1))
        big = ctx.enter_context(tc.tile_pool(name="big", bufs=1))
        st_natT = ctx.enter_context(tc.tile_pool(name="st_natT", bufs=3))
        st_selc = ctx.enter_context(tc.tile_pool(name="st_selc", bufs=3))
        st_nat = ctx.enter_context(tc.tile_pool(name="st_nat", bufs=2))
        st_msgs = ctx.enter_context(tc.tile_pool(name="st_msgs", bufs=2))
        temp1 = ctx.enter_context(tc.tile_pool(name="temp1", bufs=2))
        psM = ctx.enter_context(tc.tile_pool(name="psM", bufs=2,
                                             space="PSUM"))
        psG = ctx.enter_context(tc.tile_pool(name="psG", bufs=2,
                                             space="PSUM"))
        psC = ctx.enter_context(tc.tile_pool(name="psC", bufs=2,
                                             space="PSUM"))

        # ---------- load constants / weights ----------
        wt = {}
        for k in wnames:
            t = const.tile([D, D], bft, tag=k, name=k)
            nc.sync.dma_start(t[:], W[k][:])
            wt[k] = t
        wg1 = const.tile([D, HG], bft)
        nc.sync.dma_start(wg1[:], W["Wg1"][:])
        wg2 = const.tile([HG, 3], bft)
        nc.sync.dma_start(wg2[:], W["Wg2"][:])
        bias = {}
        for k, ap in bias_in.items():
            t = const.tile(list(ap.shape), f32, tag=k, name=k)
            nc.sync.dma_start(t[:], ap[:])
            bias[k] = t
        ident = const.tile([128, 128], bft)
        nc.sync.dma_start(ident[:], ID_d[:])
        ones2 = const.tile([2, 128], bft)
        nc.sync.dma_start(ones2[:], ONES2_d[:])
        ones3 = const.tile([3, 1], f32)
        nc.sync.dma_start(ones3[:], ONES3_d[:])
        curTb = const.tile([D, NS], bft)
        nc.sync.dma_start(curTb[:], curT_b[:])
        curTf = const.tile([D, NS], f32)
        nc.sync.dma_start(curTf[:], curT_f[:])
        bhi = const.tile([128, NSUBT * 12], bft)
        nc.sync.dma_start(bhi[:], bhi_d[:])
        invf2 = const.tile([2, NS], bft)
        nc.sync.dma_start(invf2[:], invf_d[:])

        # ---------- cpmT = Wm1.T @ curT + b_msg ----------
        cpmT = big.tile([D, NS], bft)
        for ch in range(NCHUNK):
            pm = psC.tile([128, CHUNK], f32, tag="p")
            sl = slice(ch * CHUNK, (ch + 1) * CHUNK)
            mm = nc.tensor.matmul(pm[:], wt["Wm1"][:], curTb[:, sl],
                                  start=True, stop=True)
            if ch > 0:
                mm.ins.ldweights = False
            nc.scalar.activation(cpmT[:, sl], pm[:], AF.Identity,
                                 bias=bias["b_msg"][:])

        # cpm_natA [33, 80*128]: rows 0-31 cell-major cpm per half-sb,
        # row 32 = ones (for the SELC penalty row)
        cpm_natA = big.tile([33, NHB * 128], bft)
        nc.vector.memset(cpm_natA[32:33, :], 1.0)
        for h2 in range(0, NHB, 4):
            pt = psC.tile([32, 512], bft, tag="p")
            for i in range(4):
                h = h2 + i
                nc.tensor.transpose(pt[:, i * 128:(i + 1) * 128],
                                    cpmT[:, h * 32:(h + 1) * 32], ident[:])
            nc.scalar.copy(
                cpm_natA[0:32, h2 * 128:(h2 + 4) * 128], pt[:])

        # ---------- main loop: half-superblocks ----------
        aggF32 = big.tile([128, NS], f32)     # functional agg (unnormalized)
        aggldT = big.tile([128, NSB * 128], bft)  # col t*128 + 2c+m (l,d)

        def do_half(h):
            natT_h = st_natT.tile([128, EPH], bft, tag="natT")
            nc.sync.dma_start(natT_h[:], natT_d[:, h * EPH:(h + 1) * EPH])
            selc_h = st_selc.tile([33, EPH], f8e5, tag="selc")
            nc.sync.dma_start(selc_h[:], selc_d[:, h * EPH:(h + 1) * EPH])

            # [128, 1024] so the slot is exactly 2 PSUM banks (bank-aligned)
            pmsg = psM.tile([128, 1024], f32, tag="pm")
            mm = nc.tensor.matmul(pmsg[:, 0:512], wt["Wm2"][:],
                                  natT_h[:, 0:512], start=True, stop=False)
            mm2 = nc.tensor.matmul(pmsg[:, 512:EPH], wt["Wm2"][:],
                                   natT_h[:, 512:EPH], start=True, stop=False)
            mm2.ins.ldweights = False
            stat = cpm_natA[:, h * 128:(h + 1) * 128]
            mm3 = nc.tensor.matmul(pmsg[:, 0:512], stat, selc_h[:, 0:512],
                                   start=False, stop=True)
            mm4 = nc.tensor.matmul(pmsg[:, 512:EPH], stat,
                                   selc_h[:, 512:EPH], start=False, stop=True)
            mm4.ins.ldweights = False

            msgs = st_msgs.tile([128, EPH], bft, tag="msgs")
            nc.scalar.activation(msgs[:], pmsg[:, 0:EPH], AF.Relu)
            # pairwise pre-add on gpsimd halves the DVE reduce volume
            mv = msgs[:].rearrange("p (c k) -> p c k", k=K)
            msum = st_msgs.tile([128, HCELL, 13], f32, tag="msum")
            nc.gpsimd.tensor_tensor(msum[:], mv[:, :, 0:13], mv[:, :, 13:26],
                                    OP.add)
            nc.vector.tensor_reduce(
                aggF32[:, h * HCELL:(h + 1) * HCELL], msum[:],
                AX.X, OP.add)

        def do_ld(t):
            nat_t = st_nat.tile([128, NSUB * 128], bft, tag="nat")
            nc.sync.dma_start(
                nat_t[:], nat_d[:, t * NSUB * 128:(t + 1) * NSUB * 128])
            # full-bank slot ([128, 512] f32 = 1 bank); only 128 cols used
            pagg_t = psG.tile([128, 512], f32, tag="pg")
            pagg = pagg_t[:, 0:128]
            for s in range(NSUB):
                sg = t * NSUB + s
                cb2 = 2 * CB_LOC[s]
                w = min(6, SBC - CB_LOC[s])
                nat_s = nat_t[:, s * 128:(s + 1) * 128]
                nc.tensor.matmul(pagg[:, cb2:cb2 + 2 * w], nat_s,
                                 bhi[:, sg * 12:sg * 12 + 2 * w],
                                 start=(s == 0), stop=(s == NSUB - 1))
            nc.vector.tensor_copy(aggldT[:, t * 128:(t + 1) * 128], pagg[:])

        for t in range(NSB):
            do_ld(t)
            do_half(2 * t)
            do_half(2 * t + 1)

        # ---------- normalize functional aggregate ----------
        aggFb = big.tile([128, NS], bft)
        for ch in range(NCHUNK):
            sl = slice(ch * CHUNK, (ch + 1) * CHUNK)
            pb = psC.tile([128, CHUNK], f32, tag="p")
            mm = nc.tensor.matmul(pb[:], ones2[:], invf2[:, sl], start=True,
                                  stop=True)
            if ch > 0:
                mm.ins.ldweights = False
            nc.vector.tensor_tensor(aggFb[:, sl], aggF32[:, sl], pb[:],
                                    OP.mult)

        # ---------- second stage (transposed, chunked) ----------
        localT = big.tile([128, NS], bft)
        funcT = big.tile([128, NS], bft)

        def agg_view(base_off, ch):
            # aggldT cols (t*128 + 2c + m) for cells of chunk ch
            v = aggldT[:, ch * 8 * 128 + base_off:(ch + 1) * 8 * 128:2]
            return v.rearrange("p (t c) -> p t c", c=64)

        for ch in range(NCHUNK):
            sl = slice(ch * CHUNK, (ch + 1) * CHUNK)
            pl = psC.tile([128, CHUNK], f32, tag="p")
            nc.tensor.matmul(pl[:], wt["Wl1"][:], curTb[:, sl], start=True,
                             stop=False)
            nc.tensor.matmul(
                pl[:].rearrange("p (t c) -> p t c", c=64),
                wt["Wl2"][:], agg_view(0, ch), start=False, stop=True)
            nc.scalar.activation(localT[:, sl], pl[:], AF.Tanh,
                                 bias=bias["b_local"][:])
            pf = psC.tile([128, CHUNK], f32, tag="p")
            nc.tensor.matmul(pf[:], wt["Wu1"][:], curTb[:, sl], start=True,
                             stop=False)
            nc.tensor.matmul(pf[:], wt["Wu2"][:], aggFb[:, sl],
                             start=False, stop=True)
            nc.scalar.activation(funcT[:, sl], pf[:], AF.Tanh,
                                 bias=bias["b_upd"][:])

        # CNF: 3 Euler steps
        s_prev = curTf
        s_prev_bf = curTb
        for step in range(CNF_STEPS):
            s_next = big.tile([128, NS], f32, tag=f"s{step % 2}",
                              name=f"s_next{step}")
            for ch in range(NCHUNK):
                sl = slice(ch * CHUNK, (ch + 1) * CHUNK)
                pp = psC.tile([128, CHUNK], f32, tag="p")
                nc.tensor.matmul(pp[:], wt["Wc1"][:], s_prev_bf[:, sl],
                                 start=True, stop=False)
                nc.tensor.matmul(
                    pp[:].rearrange("p (t c) -> p t c", c=64),
                    wt["Wc2"][:], agg_view(1, ch), start=False, stop=True)
                th = temp1.tile([128, CHUNK], f32, tag="th")
                nc.scalar.activation(th[:], pp[:], AF.Tanh,
                                     bias=bias["b_cnf"][:])
                nc.vector.scalar_tensor_tensor(
                    s_next[:, sl], th[:], DTC, s_prev[:, sl],
                    OP.mult, OP.add)
            s_prev = s_next
            if step < CNF_STEPS - 1:
                nb = big.tile([128, NS], bft, tag="sbf", name=f"sbf{step}")
                nc.vector.tensor_copy(nb[:], s_next[:])
                s_prev_bf = nb

        # ---------- gating + final mix, fused per chunk ----------
        for ch in range(NCHUNK):
            sl = slice(ch * CHUNK, (ch + 1) * CHUNK)
            ph = psC.tile([HG, CHUNK], f32, tag="p")
            nc.tensor.matmul(ph[:], wg1[:], curTb[:, sl], start=True,
                             stop=True)
            hT = temp1.tile([HG, CHUNK], bft, tag="hT")
            nc.scalar.activation(hT[:], ph[:], AF.Relu, bias=bias["b_g1"][:])
            pz = psC.tile([3, CHUNK], f32, tag="p")
            nc.tensor.matmul(pz[:], wg2[:], hT[:], start=True, stop=True)
            e3 = temp1.tile([3, CHUNK], f32, tag="e3")
            nc.scalar.activation(e3[:], pz[:], AF.Exp, bias=bias["b_g2"][:])
            p1_t = psG.tile([128, 512], f32, tag="pg")
            p1 = p1_t[0:1, 0:CHUNK]
            nc.tensor.matmul(p1[:], ones3[:], e3[:], start=True, stop=True)
            rec = temp1.tile([1, CHUNK], f32, tag="rec")
            nc.vector.reciprocal_approx_fast(rec[:], p1[:])
            rbc = temp1.tile([128, CHUNK], f32, tag="rbc")
            nc.gpsimd.partition_broadcast(rbc[:], rec[:])
            ge = []
            for m in range(3):
                # partition_broadcast needs its input on partition 0; DMA the
                # gate row down from partition m first
                erow = temp1.tile([1, CHUNK], f32, tag=f"erow{m}",
                                  name=f"erow{m}")
                nc.sync.dma_start(erow[:], e3[m:m + 1, :])
                g = temp1.tile([128, CHUNK], f32, tag=f"ge{m}",
                               name=f"ge{m}")
                nc.gpsimd.partition_broadcast(g[:], erow[:])
                ge.append(g)
            acc = temp1.tile([128, CHUNK], f32, tag="acc")
            tmp = temp1.tile([128, CHUNK], f32, tag="tmp")
            accb = temp1.tile([128, CHUNK], bft, tag="accb")
            nc.vector.tensor_tensor(acc[:], localT[:, sl], ge[0][:], OP.mult)
            nc.vector.tensor_tensor(tmp[:], funcT[:, sl], ge[1][:], OP.mult)
            nc.vector.tensor_tensor(acc[:], acc[:], tmp[:], OP.add)
            nc.vector.tensor_tensor(tmp[:], s_prev[:, sl], ge[2][:], OP.mult)
            nc.vector.tensor_tensor(acc[:], acc[:], tmp[:], OP.add)
            nc.vector.tensor_tensor(accb[:], acc[:], rbc[:], OP.mult)
            nc.sync.dma_start(outT[:, sl], accb[:])

    nc.compile()
    return nc


_NC_CACHE = None


def _get_nc():
    global _NC_CACHE
    if _NC_CACHE is None:
        _NC_CACHE = _build_bass()
    return _NC_CACHE


def _split_hilo(w):
    hi = w.astype(bf16)
    lo = (w - hi.astype(np.float32)).astype(bf16)
    return hi, lo


def _prep_core_inputs(cur, nbr, conn, weights):
    """cur [NS, D] f32, nbr [NS, K, D] f32, conn [NS, K] i32 -> input map."""
    m = {}
    x = nbr.reshape(E, D).astype(bf16)
    m["natT"] = np.ascontiguousarray(x.T)                       # [128, E]
    m["nat"] = np.ascontiguousarray(
        x.reshape(NSUBT, 128, D).transpose(1, 0, 2)).reshape(128, NSUBT * D)

    cf = conn.reshape(E)
    mf = (cf == 1).astype(np.float32)
    ml = (cf == 0).astype(np.float32)
    md = (cf == 2).astype(np.float32)

    # SELC: masked staircase + penalty row (fp8e5: 0/1/-32768 all exact)
    f8 = ml_dtypes.float8_e5m2
    selc = np.zeros((33, E), f8)
    eidx = np.arange(E)
    cl32 = (eidx // K) % HCELL
    selc[cl32, eidx] = mf.astype(f8)
    selc[32, :] = (PEN * (1.0 - mf)).astype(f8)
    m["SELC"] = selc

    # per-cell inverse counts
    cnt_l = ml.reshape(NS, K).sum(1)
    cnt_f = mf.reshape(NS, K).sum(1)
    cnt_d = md.reshape(NS, K).sum(1)
    inv_l = 1.0 / np.maximum(cnt_l, 1.0)
    inv_f = 1.0 / np.maximum(cnt_f, 1.0)
    inv_d = 1.0 / np.maximum(cnt_d, 1.0)

    # B_hi/B_lo: staircase * mask * inv, interleaved (l,d) per cell
    cell = eidx // K          # global cell per edge
    cl64 = cell % SBC         # local cell within superblock
    s_of_e = eidx // 128      # subtile
    j = cl64 - np.asarray(CB_LOC)[s_of_e % NSUB]   # 0..5
    p_of_e = eidx % 128
    w_l = ml * inv_l[cell]
    w_d = md * inv_d[cell]
    B = np.zeros((128, NSUBT * 12), np.float32)
    B[p_of_e, s_of_e * 12 + 2 * j] = w_l
    B[p_of_e, s_of_e * 12 + 2 * j + 1] = w_d
    m["B_hi"] = B.astype(bf16)

    ihi, ilo = _split_hilo(inv_f.astype(np.float32))
    m["INVF2"] = np.stack([ihi, ilo], axis=0)                   # [2, NS]

    ct = np.ascontiguousarray(cur.T)
    m["curT_f"] = ct.astype(np.float32)
    m["curT_b"] = ct.astype(bf16)

    Wl, Wm, Wu, Wc = (weights["W_local"], weights["W_msg"],
                      weights["W_upd"], weights["W_cnf"])
    m["Wl1"], m["Wl2"] = Wl[:D].astype(bf16), Wl[D:].astype(bf16)
    m["Wm1"], m["Wm2"] = Wm[:D].astype(bf16), Wm[D:].astype(bf16)
    m["Wu1"], m["Wu2"] = Wu[:D].astype(bf16), Wu[D:].astype(bf16)
    m["Wc1"], m["Wc2"] = Wc[:D].astype(bf16), Wc[D:].astype(bf16)
    m["Wg1"] = weights["W_g1"].astype(bf16)
    m["Wg2"] = weights["W_g2"].astype(bf16)
    m["b_local"] = weights["b_local"].reshape(D, 1).astype(np.float32)
    m["b_upd"] = weights["b_upd"].reshape(D, 1).astype(np.float32)
    m["b_cnf"] = weights["b_cnf"].reshape(D, 1).astype(np.float32)
    m["b_msg"] = weights["b_msg"].reshape(D, 1).astype(np.float32)
    m["b_g1"] = weights["b_g1"].reshape(HG, 1).astype(np.float32)
    m["b_g2"] = weights["b_g2"].reshape(3, 1).astype(np.float32)
    for k, v in CONSTS.items():
        m[k] = v
    return m


def kernel(**inputs):
    from concourse.bass_utils import run_bass_kernel_spmd

    cur = np.asarray(inputs["current_state"], np.float32)
    nbr = np.asarray(inputs["neighbor_states"], np.float32)
    conn = np.asarray(inputs["conn_type"], np.int32)
    weights = {k: np.asarray(v, np.float32) for k, v in inputs.items()
               if k not in ("current_state", "neighbor_states", "conn_type")}

    npad = NCORES * NS
    cur_p = np.zeros((npad, D), np.float32)
    cur_p[:N_CELLS] = cur
    nbr_p = np.zeros((npad, K, D), np.float32)
    nbr_p[:N_CELLS] = nbr
    conn_p = np.full((npad, K), 3, np.int32)
    conn_p[:N_CELLS] = conn

    in_maps = []
    for c in range(NCORES):
        sl = slice(c * NS, (c + 1) * NS)
        in_maps.append(_prep_core_inputs(cur_p[sl], nbr_p[sl], conn_p[sl],
                                         weights))
    nc = _get_nc()
    res = run_bass_kernel_spmd(nc, in_maps, list(range(NCORES)))
    out = np.concatenate([res.results[c]["outT"].T for c in range(NCORES)],
                         axis=0)
    return np.ascontiguousarray(out[:N_CELLS]).astype(np.float32)


if __name__ == "__main__":
    pass
